# revision 9
# baseline (speedup 1.0000x reference)
"""Trainium2 Bass kernel for nn_Decoder (attention GRU decoder step).

Strategy (8 NeuronCores, zero collectives — collectives cost ~75us+ in this
environment, far above their data cost):

  Launch 1 (batch-parallel, 8 batches/core):
      dot-attention (scores via DVE mul+reduce, softmax, context via PE)
      + GRU cell (PE matmuls, gates on DVE/ACT).
      Outputs per core: h_new rows [8, 512], context^T [512, 8].
  Host: assembles x2h^T = [h_new; context]^T  (tiny: 256 KB).
  Launch 2 (vocab-parallel, 4000 vocab rows/core):
      logits slice [64, 4000] = x2h @ W_out_slice^T + b_out_slice,
      streamed fp32r matmuls against the 16.4 MB/core weight slice.

All heavy compute runs on device; the host only shards/reassembles and
pre-transposes weights (layout prep, done once per call).
"""

import os
import numpy as np

B, S, H, EMB, V = 64, 128, 512, 512, 32000
NC = 8
BL = B // NC          # 8 local batches per core
VL = V // NC          # 4000 vocab rows per core
NB = 500              # psum bank chunk of the vocab slice (8 * 500 = 4000)
KX = (EMB + H) // 128  # 8 k-chunks of x
KH = H // 128          # 4 k-chunks of h
G3 = 3 * H             # 1536 gate units

_cache = {}


def _get(key, builder):
    if key not in _cache:
        _cache[key] = builder()
    return _cache[key]


def _build_l1():
    from contextlib import ExitStack
    from concourse import bacc, tile, mybir
    from concourse.masks import make_identity

    f32 = mybir.dt.float32
    f32r = mybir.dt.float32r
    AT = mybir.ActivationFunctionType
    OP = mybir.AluOpType
    AX = mybir.AxisListType

    nc = bacc.Bacc("TRN2", target_bir_lowering=False, debug=False,
                   num_devices=NC)
    enc = nc.dram_tensor("enc", [BL, S, H], f32, kind="ExternalInput").ap()
    q_in = nc.dram_tensor("q", [BL, H], f32, kind="ExternalInput").ap()
    msk = nc.dram_tensor("msk", [BL, S], f32, kind="ExternalInput").ap()
    emb = nc.dram_tensor("emb", [BL, EMB], f32, kind="ExternalInput").ap()
    wih = nc.dram_tensor("wih", [EMB + H, G3], f32, kind="ExternalInput").ap()
    whh = nc.dram_tensor("whh", [H, G3], f32, kind="ExternalInput").ap()
    bih = nc.dram_tensor("bih", [1, G3], f32, kind="ExternalInput").ap()
    bhh = nc.dram_tensor("bhh", [1, G3], f32, kind="ExternalInput").ap()
    hn_o = nc.dram_tensor("hn", [BL, H], f32, kind="ExternalOutput").ap()
    cxT_o = nc.dram_tensor("cxT", [H, BL], f32, kind="ExternalOutput").ap()

    with tile.TileContext(nc) as tc, ExitStack() as ctx:
        const = ctx.enter_context(tc.tile_pool(name="const", bufs=1))
        acts = ctx.enter_context(tc.tile_pool(name="acts", bufs=1))
        wp = ctx.enter_context(tc.tile_pool(name="wp", bufs=1))
        bc = ctx.enter_context(tc.tile_pool(name="bc", bufs=2))
        ps = ctx.enter_context(tc.tile_pool(name="ps", bufs=2, space="PSUM"))
        psg = ctx.enter_context(tc.tile_pool(name="psg", bufs=1, space="PSUM"))

        ident = const.tile([128, 128], f32, tag="ident")
        make_identity(nc, ident[:])
        ones18 = const.tile([1, BL], f32, tag="ones18")
        nc.vector.memset(ones18[:], 1.0)

        enc_sb = []
        for b in range(BL):
            t = acts.tile([S, H], f32, tag=f"enc{b}")
            nc.sync.dma_start(t[:], enc[b, :, :])
            enc_sb.append(t)
        q_sb = acts.tile([BL, H], f32, tag="q")
        nc.sync.dma_start(q_sb[:], q_in[:])
        msk_sb = acts.tile([BL, S], f32, tag="msk")
        nc.sync.dma_start(msk_sb[:], msk[:])
        emb_sb = acts.tile([BL, EMB], f32, tag="emb")
        nc.sync.dma_start(emb_sb[:], emb[:])
        wih_sb = wp.tile([128, KX * G3], f32, tag="wih")
        for k in range(KX):
            nc.sync.dma_start(wih_sb[:, k * G3:(k + 1) * G3],
                              wih[k * 128:(k + 1) * 128, :])
        whh_sb = wp.tile([128, KH * G3], f32, tag="whh")
        for k in range(KH):
            nc.sync.dma_start(whh_sb[:, k * G3:(k + 1) * G3],
                              whh[k * 128:(k + 1) * 128, :])
        bih_sb = const.tile([1, G3], f32, tag="bih")
        nc.sync.dma_start(bih_sb[:], bih[:])
        bhh_sb = const.tile([1, G3], f32, tag="bhh")
        nc.sync.dma_start(bhh_sb[:], bhh[:])

        # ---- scores^T [128s, 8b] via q-broadcast + fused mul-reduce ----
        scT = acts.tile([S, BL], f32, tag="scT")
        for b in range(BL):
            qrow = bc.tile([1, H], f32, tag="qrow", name=f"qrow{b}")
            nc.sync.dma_start(qrow[:], q_in[b:b + 1, :])
            qb = bc.tile([S, H], f32, tag="qb")
            nc.gpsimd.partition_broadcast(qb[:], qrow[:])
            prod = bc.tile([S, H], f32, tag="prod")
            nc.vector.tensor_mul(prod[:], enc_sb[b][:], qb[:])
            nc.vector.reduce_sum(scT[:, b:b + 1], prod[:], axis=AX.X)

        # ---- transpose scores -> [8b, 128s], mask, softmax ----
        sc_ps = ps.tile([BL, S], f32, tag="pst")
        nc.tensor.transpose(sc_ps[:], scT[:], ident[:])
        sc = acts.tile([BL, S], f32, tag="sc")
        nc.vector.tensor_copy(sc[:], sc_ps[:])
        t1 = acts.tile([BL, S], f32, tag="t1")
        nc.vector.tensor_mul(t1[:], sc[:], msk_sb[:])
        t2 = acts.tile([BL, S], f32, tag="t2")
        nc.vector.tensor_scalar(out=t2[:], in0=msk_sb[:], scalar1=1.0,
                                scalar2=100000.0, op0=OP.subtract,
                                op1=OP.mult)
        tm = acts.tile([BL, S], f32, tag="tm")
        nc.vector.tensor_add(tm[:], t1[:], t2[:])
        rmax = acts.tile([BL, 1], f32, tag="rmax")
        nc.vector.reduce_max(rmax[:], tm[:], axis=AX.X)
        nmax = acts.tile([BL, 1], f32, tag="nmax")
        nc.vector.tensor_scalar_mul(nmax[:], rmax[:], -1.0)
        esb = acts.tile([BL, S], f32, tag="esb")
        rsum = acts.tile([BL, 1], f32, tag="rsum")
        nc.scalar.activation(esb[:], tm[:], AT.Exp, bias=nmax[:, 0:1],
                             scale=1.0, accum_out=rsum[:])
        rinv = acts.tile([BL, 1], f32, tag="rinv")
        nc.vector.reciprocal(rinv[:], rsum[:])
        wgt = acts.tile([BL, S], f32, tag="wgt")
        nc.vector.tensor_scalar_mul(wgt[:], esb[:], rinv[:, 0:1])
        wT_ps = ps.tile([S, BL], f32, tag="pst")
        nc.tensor.transpose(wT_ps[:], wgt[:], ident[:BL, :BL])
        wT = acts.tile([S, BL], f32, tag="wT")
        nc.vector.tensor_copy(wT[:], wT_ps[:])

        # ---- context^T [512, 8] : ctxT[h,b] = sum_s enc[b,s,h] * w[b,s] ----
        cxT_sb = acts.tile([128, KH * BL], f32, tag="cxT")
        for c in range(KH):
            cp = ps.tile([128, BL], f32, tag="pst")
            for b in range(BL):
                nc.tensor.matmul(cp[:, b:b + 1],
                                 lhsT=enc_sb[b][:, c * 128:(c + 1) * 128],
                                 rhs=wT[:, b:b + 1],
                                 start=True, stop=True)
            nc.vector.tensor_copy(cxT_sb[:, c * BL:(c + 1) * BL], cp[:])
            nc.sync.dma_start(cxT_o[c * 128:(c + 1) * 128, :],
                              cxT_sb[:, c * BL:(c + 1) * BL])

        # ---- transpose emb and h(=q) to [k, b] chunks for GRU lhsT ----
        embT = acts.tile([128, KH * BL], f32, tag="embT")
        hT = acts.tile([128, KH * BL], f32, tag="hT")
        for c in range(KH):
            p1 = ps.tile([128, BL], f32, tag="pst")
            nc.tensor.transpose(p1[:], emb_sb[:, c * 128:(c + 1) * 128],
                                ident[:BL, :BL])
            nc.vector.tensor_copy(embT[:, c * BL:(c + 1) * BL], p1[:])
            p2 = ps.tile([128, BL], f32, tag="pst")
            nc.tensor.transpose(p2[:], q_sb[:, c * 128:(c + 1) * 128],
                                ident[:BL, :BL])
            nc.vector.tensor_copy(hT[:, c * BL:(c + 1) * BL], p2[:])

        # ---- GRU gates: gx = x @ W_ih^T + b_ih ; gh = h @ W_hh^T + b_hh ----
        gx = psg.tile([BL, G3], f32, tag="gx")
        gh = psg.tile([BL, G3], f32, tag="gh")
        for c in range(3):
            nc.tensor.matmul(gx[:, c * 512:(c + 1) * 512], lhsT=ones18[:],
                             rhs=bih_sb[:, c * 512:(c + 1) * 512],
                             start=True, stop=False)
            nc.tensor.matmul(gh[:, c * 512:(c + 1) * 512], lhsT=ones18[:],
                             rhs=bhh_sb[:, c * 512:(c + 1) * 512],
                             start=True, stop=False)
        for k in range(KX):
            src = embT if k < KH else cxT_sb
            kk = k if k < KH else k - KH
            lhs = src[:, kk * BL:(kk + 1) * BL]
            for c in range(3):
                nc.tensor.matmul(
                    gx[:, c * 512:(c + 1) * 512],
                    lhsT=lhs,
                    rhs=wih_sb[:, k * G3 + c * 512:k * G3 + (c + 1) * 512],
                    start=False, stop=(k == KX - 1))
        for k in range(KH):
            lhs = hT[:, k * BL:(k + 1) * BL]
            for c in range(3):
                nc.tensor.matmul(
                    gh[:, c * 512:(c + 1) * 512],
                    lhsT=lhs,
                    rhs=whh_sb[:, k * G3 + c * 512:k * G3 + (c + 1) * 512],
                    start=False, stop=(k == KH - 1))

        # ---- gate nonlinearities (PyTorch order r, z, n) ----
        gx_sb = acts.tile([BL, G3], f32, tag="gxsb")
        nc.vector.tensor_copy(gx_sb[:], gx[:])
        r_in = acts.tile([BL, H], f32, tag="rin")
        nc.vector.tensor_add(r_in[:], gx_sb[:, 0:H], gh[:, 0:H])
        r_sb = acts.tile([BL, H], f32, tag="rsb")
        nc.scalar.activation(r_sb[:], r_in[:], AT.Sigmoid)
        z_in = acts.tile([BL, H], f32, tag="zin")
        nc.vector.tensor_add(z_in[:], gx_sb[:, H:2 * H], gh[:, H:2 * H])
        z_sb = acts.tile([BL, H], f32, tag="zsb")
        nc.scalar.activation(z_sb[:], z_in[:], AT.Sigmoid)
        nmul = acts.tile([BL, H], f32, tag="nmul")
        nc.vector.tensor_mul(nmul[:], r_sb[:], gh[:, 2 * H:3 * H])
        n_in = acts.tile([BL, H], f32, tag="nin")
        nc.vector.tensor_add(n_in[:], nmul[:], gx_sb[:, 2 * H:3 * H])
        n_sb = acts.tile([BL, H], f32, tag="nsb")
        nc.scalar.activation(n_sb[:], n_in[:], AT.Tanh)
        # h_new = (1-z)*n + z*h = n + z*(h-n)
        d_sb = acts.tile([BL, H], f32, tag="dsb")
        nc.vector.tensor_sub(d_sb[:], q_sb[:], n_sb[:])
        zd = acts.tile([BL, H], f32, tag="zd")
        nc.vector.tensor_mul(zd[:], z_sb[:], d_sb[:])
        hnew = acts.tile([BL, H], f32, tag="hnew")
        nc.vector.tensor_add(hnew[:], n_sb[:], zd[:])
        nc.sync.dma_start(hn_o[:], hnew[:])

    nc.compile()
    return nc


def _build_l2():
    from contextlib import ExitStack
    from concourse import bacc, tile, mybir

    f32 = mybir.dt.float32
    bf16 = mybir.dt.bfloat16

    nc = bacc.Bacc("TRN2", target_bir_lowering=False, debug=False,
                   num_devices=NC)
    x2h = nc.dram_tensor("x2h", [EMB + H, B], bf16, kind="ExternalInput").ap()
    x2l = nc.dram_tensor("x2l", [EMB + H, B], bf16, kind="ExternalInput").ap()
    wth = nc.dram_tensor("wth", [EMB + H, VL], bf16, kind="ExternalInput").ap()
    wtl = nc.dram_tensor("wtl", [EMB + H, VL], bf16, kind="ExternalInput").ap()
    bo = nc.dram_tensor("bo", [1, VL], f32, kind="ExternalInput").ap()
    lg = nc.dram_tensor("lg", [B, VL], f32, kind="ExternalOutput").ap()

    with tile.TileContext(nc) as tc, ExitStack() as ctx:
        const = ctx.enter_context(tc.tile_pool(name="const", bufs=1))
        xp = ctx.enter_context(tc.tile_pool(name="xp", bufs=1))
        wp = ctx.enter_context(tc.tile_pool(name="wp", bufs=1))
        op = ctx.enter_context(tc.tile_pool(name="op", bufs=2))
        psp = ctx.enter_context(tc.tile_pool(name="psp", bufs=1, space="PSUM"))

        ones = const.tile([1, B], f32, tag="ones")
        nc.vector.memset(ones[:], 1.0)
        bo_sb = const.tile([1, VL], f32, tag="bo")
        nc.sync.dma_start(bo_sb[:], bo[:])
        x2h_sb = xp.tile([128, KX * B], bf16, tag="x2h")
        x2l_sb = xp.tile([128, KX * B], bf16, tag="x2l")
        for k in range(KX):
            nc.sync.dma_start(x2h_sb[:, k * B:(k + 1) * B],
                              x2h[k * 128:(k + 1) * 128, :])
            nc.sync.dma_start(x2l_sb[:, k * B:(k + 1) * B],
                              x2l[k * 128:(k + 1) * 128, :])
        wth_sb, wtl_sb = [], []
        for k in range(KX):
            th = wp.tile([128, VL], bf16, tag=f"wth{k}", name=f"wth{k}")
            nc.sync.dma_start(th[:], wth[k * 128:(k + 1) * 128, :])
            wth_sb.append(th)
            tl = wp.tile([128, VL], bf16, tag=f"wtl{k}", name=f"wtl{k}")
            nc.sync.dma_start(tl[:], wtl[k * 128:(k + 1) * 128, :])
            wtl_sb.append(tl)

        lgp = [psp.tile([B, NB], f32, tag=f"lg{nb}", name=f"lgp{nb}")
               for nb in range(VL // NB)]
        for nb in range(VL // NB):
            nc.tensor.matmul(lgp[nb][:], lhsT=ones[:],
                             rhs=bo_sb[:, nb * NB:(nb + 1) * NB],
                             start=True, stop=False)
        for k in range(KX):
            for nb in range(VL // NB):
                s = slice(nb * NB, (nb + 1) * NB)
                last = (k == KX - 1)
                nc.tensor.matmul(lgp[nb][:],
                                 lhsT=x2h_sb[:, k * B:(k + 1) * B],
                                 rhs=wth_sb[k][:, s], start=False, stop=False)
                nc.tensor.matmul(lgp[nb][:],
                                 lhsT=x2l_sb[:, k * B:(k + 1) * B],
                                 rhs=wth_sb[k][:, s], start=False, stop=False)
                nc.tensor.matmul(lgp[nb][:],
                                 lhsT=x2h_sb[:, k * B:(k + 1) * B],
                                 rhs=wtl_sb[k][:, s], start=False, stop=last)
        for nb in range(VL // NB):
            o = op.tile([B, NB], f32, tag="o")
            nc.vector.tensor_copy(o[:], lgp[nb][:])
            nc.sync.dma_start(lg[:, nb * NB:(nb + 1) * NB], o[:])

    nc.compile()
    return nc


last_exec_times = []


def _install_ntff_hook():
    """Shim antenv.axon_hooks so trace=True captures NTFF under axon."""
    import sys, types
    try:
        import antenv.axon_hooks  # noqa: F401
        return
    except ImportError:
        pass
    try:
        import antenv
        from trn_agent_boot.trn_boot import _ntff_profile_via_ctypes
        mod = types.ModuleType("antenv.axon_hooks")
        _store = {}
        mod.set_axon_ntff_profile_hook = lambda h: _store.update(h=h)
        mod.get_axon_ntff_profile_hook = lambda: _store.get("h")
        sys.modules["antenv.axon_hooks"] = mod
        antenv.axon_hooks = mod
        mod.set_axon_ntff_profile_hook(
            _ntff_profile_via_ctypes("/opt/axon/libaxon_pjrt.so"))
    except Exception:
        pass


def _run(nc, in_maps):
    from concourse import bass_utils
    trace = bool(int(os.environ.get("BASSDEC_TRACE", "0")))
    if trace:
        _install_ntff_hook()
    res = bass_utils.run_bass_kernel_spmd(
        nc, in_maps, core_ids=list(range(NC)), trace=trace)
    if trace:
        last_exec_times.append(res.exec_time_ns)
    return res.results


def kernel(input_token, hidden, enc_out, src_mask, emb_table,
           W_ih, W_hh, b_ih, b_hh, W_out, b_out):
    f = np.float32
    input_token = np.asarray(input_token)
    hidden = np.asarray(hidden, f)
    enc_out = np.ascontiguousarray(np.asarray(enc_out, f))
    src_mask_f = np.asarray(src_mask).astype(f)
    emb_table = np.asarray(emb_table, f)
    W_ih = np.asarray(W_ih, f)
    W_hh = np.asarray(W_hh, f)
    b_ih = np.asarray(b_ih, f)
    b_hh = np.asarray(b_hh, f)
    W_out = np.asarray(W_out, f)
    b_out = np.asarray(b_out, f)

    l1 = _get("l1", _build_l1)
    l2 = _get("l2", _build_l2)

    tokens = input_token.astype(np.int64)
    emb_rows = np.ascontiguousarray(emb_table[tokens])          # [64, 512]
    wihT = np.ascontiguousarray(W_ih.T)                          # [1024, 1536]
    whhT = np.ascontiguousarray(W_hh.T)                          # [512, 1536]
    bih_r = np.ascontiguousarray(b_ih.reshape(1, G3))
    bhh_r = np.ascontiguousarray(b_hh.reshape(1, G3))
    q = hidden[0]                                                # [64, 512]

    in_maps1 = []
    for c in range(NC):
        sl = slice(c * BL, (c + 1) * BL)
        in_maps1.append({
            "enc": np.ascontiguousarray(enc_out[sl]),
            "q": np.ascontiguousarray(q[sl]),
            "msk": np.ascontiguousarray(src_mask_f[sl]),
            "emb": np.ascontiguousarray(emb_rows[sl]),
            "wih": wihT, "whh": whhT, "bih": bih_r, "bhh": bhh_r,
        })
    res1 = _run(l1, in_maps1)

    import ml_dtypes
    bf16 = ml_dtypes.bfloat16
    h_new = np.concatenate([res1[c]["hn"] for c in range(NC)], axis=0)
    ctx = np.concatenate([res1[c]["cxT"].T for c in range(NC)], axis=0)
    x2 = np.concatenate([h_new, ctx], axis=1)                    # [64, 1024]
    x2T = np.ascontiguousarray(x2.T)                             # [1024, 64]
    x2T_hi = x2T.astype(bf16)
    x2T_lo = (x2T - x2T_hi.astype(f)).astype(bf16)
    W_outT = np.ascontiguousarray(W_out.T)                       # [1024, 32000]
    WT_hi = W_outT.astype(bf16)
    WT_lo = (W_outT - WT_hi.astype(f)).astype(bf16)

    in_maps2 = []
    for c in range(NC):
        vs = slice(c * VL, (c + 1) * VL)
        in_maps2.append({
            "x2h": x2T_hi, "x2l": x2T_lo,
            "wth": np.ascontiguousarray(WT_hi[:, vs]),
            "wtl": np.ascontiguousarray(WT_lo[:, vs]),
            "bo": np.ascontiguousarray(b_out[vs].reshape(1, VL)),
        })
    res2 = _run(l2, in_maps2)

    logits = np.concatenate([res2[c]["lg"] for c in range(NC)], axis=1)
    return logits.astype(f), h_new[None].astype(f)


# revision 12
# speedup vs baseline: 1.4185x; 1.4185x over previous
"""Trainium2 Bass kernel for nn_Decoder (attention GRU decoder step).

Strategy (8 NeuronCores, zero collectives — collectives cost ~75us+ in this
environment, far above their data cost):

  Launch 1 (batch-parallel, 8 batches/core):
      dot-attention (scores via DVE mul+reduce, softmax, context via PE)
      + GRU cell (PE matmuls, gates on DVE/ACT).
      Outputs per core: h_new rows [8, 512], context^T [512, 8].
  Host: assembles x2h^T = [h_new; context]^T  (tiny: 256 KB).
  Launch 2 (vocab-parallel, 4000 vocab rows/core):
      logits slice [64, 4000] = x2h @ W_out_slice^T + b_out_slice,
      streamed fp32r matmuls against the 16.4 MB/core weight slice.

All heavy compute runs on device; the host only shards/reassembles and
pre-transposes weights (layout prep, done once per call).
"""

import os
import numpy as np

B, S, H, EMB, V = 64, 128, 512, 512, 32000
NC = 8
BL = B // NC          # 8 local batches per core
VL = V // NC          # 4000 vocab rows per core
NB = 500              # psum bank chunk of the vocab slice (8 * 500 = 4000)
KX = (EMB + H) // 128  # 8 k-chunks of x
KH = H // 128          # 4 k-chunks of h
G3 = 3 * H             # 1536 gate units

_cache = {}


def _get(key, builder):
    if key not in _cache:
        _cache[key] = builder()
    return _cache[key]


def _build_l1():
    from contextlib import ExitStack
    from concourse import bacc, tile, mybir
    from concourse.masks import make_identity

    f32 = mybir.dt.float32
    f32r = mybir.dt.float32r
    AT = mybir.ActivationFunctionType
    OP = mybir.AluOpType
    AX = mybir.AxisListType

    nc = bacc.Bacc("TRN2", target_bir_lowering=False, debug=False,
                   num_devices=NC)
    enc = nc.dram_tensor("enc", [BL, S, H], f32, kind="ExternalInput").ap()
    q_in = nc.dram_tensor("q", [BL, H], f32, kind="ExternalInput").ap()
    msk = nc.dram_tensor("msk", [BL, S], f32, kind="ExternalInput").ap()
    emb = nc.dram_tensor("emb", [BL, EMB], f32, kind="ExternalInput").ap()
    wih = nc.dram_tensor("wih", [EMB + H, G3], f32r, kind="ExternalInput").ap()
    whh = nc.dram_tensor("whh", [H, G3], f32r, kind="ExternalInput").ap()
    bih = nc.dram_tensor("bih", [1, G3], f32r, kind="ExternalInput").ap()
    bhh = nc.dram_tensor("bhh", [1, G3], f32r, kind="ExternalInput").ap()
    hn_o = nc.dram_tensor("hn", [BL, H], f32, kind="ExternalOutput").ap()
    cxT_o = nc.dram_tensor("cxT", [H, BL], f32, kind="ExternalOutput").ap()

    with tile.TileContext(nc) as tc, ExitStack() as ctx:
        const = ctx.enter_context(tc.tile_pool(name="const", bufs=1))
        acts = ctx.enter_context(tc.tile_pool(name="acts", bufs=1))
        wp = ctx.enter_context(tc.tile_pool(name="wp", bufs=1))
        bc = ctx.enter_context(tc.tile_pool(name="bc", bufs=2))
        ps = ctx.enter_context(tc.tile_pool(name="ps", bufs=2, space="PSUM"))
        psg = ctx.enter_context(tc.tile_pool(name="psg", bufs=1, space="PSUM"))

        ident = const.tile([128, 128], f32, tag="ident")
        make_identity(nc, ident[:])
        ones18f = const.tile([1, BL], f32, tag="ones18f")
        nc.vector.memset(ones18f[:], 1.0)
        ones18 = const.tile([1, BL], f32r, tag="ones18")
        nc.vector.tensor_copy(ones18[:], ones18f[:])

        # small activation DMAs first so attention starts immediately;
        # big GRU-weight DMAs are emitted last (they are only needed ~30us in)
        q_sb = acts.tile([BL, H], f32, tag="q")
        nc.sync.dma_start(q_sb[:], q_in[:])
        msk_sb = acts.tile([BL, S], f32, tag="msk")
        nc.sync.dma_start(msk_sb[:], msk[:])
        emb_sb = acts.tile([BL, EMB], f32, tag="emb")
        nc.sync.dma_start(emb_sb[:], emb[:])
        bih_sb = const.tile([1, G3], f32r, tag="bih")
        nc.sync.dma_start(bih_sb[:], bih[:])
        bhh_sb = const.tile([1, G3], f32r, tag="bhh")
        nc.sync.dma_start(bhh_sb[:], bhh[:])
        qrows = []
        for b in range(BL):
            qrow = bc.tile([1, H], f32, tag="qrow", name=f"qrow{b}")
            nc.sync.dma_start(qrow[:], q_in[b:b + 1, :])
            qrows.append(qrow)
        enc_sb = []
        for b in range(BL):
            t = acts.tile([S, H], f32, tag=f"enc{b}")
            nc.sync.dma_start(t[:], enc[b, :, :])
            enc_sb.append(t)
        wih_sb = wp.tile([128, KX * G3], f32r, tag="wih")
        for k in range(KX):
            nc.sync.dma_start(wih_sb[:, k * G3:(k + 1) * G3],
                              wih[k * 128:(k + 1) * 128, :])
        whh_sb = wp.tile([128, KH * G3], f32r, tag="whh")
        for k in range(KH):
            nc.sync.dma_start(whh_sb[:, k * G3:(k + 1) * G3],
                              whh[k * 128:(k + 1) * 128, :])

        # ---- scores^T [128s, 8b] via q-broadcast + fused mul-reduce ----
        scT = acts.tile([S, BL], f32, tag="scT")
        for b in range(BL):
            qb = bc.tile([S, H], f32, tag="qb")
            nc.gpsimd.partition_broadcast(qb[:], qrows[b][:])
            prod = bc.tile([S, H], f32, tag="prod")
            nc.vector.tensor_mul(prod[:], enc_sb[b][:], qb[:])
            nc.vector.reduce_sum(scT[:, b:b + 1], prod[:], axis=AX.X)

        # ---- transpose scores -> [8b, 128s], mask, softmax ----
        sc_ps = ps.tile([BL, S], f32, tag="pst")
        nc.tensor.transpose(sc_ps[:], scT[:], ident[:])
        sc = acts.tile([BL, S], f32, tag="sc")
        nc.vector.tensor_copy(sc[:], sc_ps[:])
        t1 = acts.tile([BL, S], f32, tag="t1")
        nc.vector.tensor_mul(t1[:], sc[:], msk_sb[:])
        t2 = acts.tile([BL, S], f32, tag="t2")
        nc.vector.tensor_scalar(out=t2[:], in0=msk_sb[:], scalar1=1.0,
                                scalar2=100000.0, op0=OP.subtract,
                                op1=OP.mult)
        tm = acts.tile([BL, S], f32, tag="tm")
        nc.vector.tensor_add(tm[:], t1[:], t2[:])
        rmax = acts.tile([BL, 1], f32, tag="rmax")
        nc.vector.reduce_max(rmax[:], tm[:], axis=AX.X)
        nmax = acts.tile([BL, 1], f32, tag="nmax")
        nc.vector.tensor_scalar_mul(nmax[:], rmax[:], -1.0)
        esb = acts.tile([BL, S], f32, tag="esb")
        rsum = acts.tile([BL, 1], f32, tag="rsum")
        nc.scalar.activation(esb[:], tm[:], AT.Exp, bias=nmax[:, 0:1],
                             scale=1.0, accum_out=rsum[:])
        rinv = acts.tile([BL, 1], f32, tag="rinv")
        nc.vector.reciprocal(rinv[:], rsum[:])
        wgt = acts.tile([BL, S], f32, tag="wgt")
        nc.vector.tensor_scalar_mul(wgt[:], esb[:], rinv[:, 0:1])
        wT_ps = ps.tile([S, BL], f32, tag="pst")
        nc.tensor.transpose(wT_ps[:], wgt[:], ident[:BL, :BL])
        wT = acts.tile([S, BL], f32, tag="wT")
        nc.vector.tensor_copy(wT[:], wT_ps[:])

        # ---- context^T [512, 8] : ctxT[h,b] = sum_s enc[b,s,h] * w[b,s] ----
        cxT_sb = acts.tile([128, KH * BL], f32r, tag="cxT")
        for c in range(KH):
            cp = ps.tile([128, BL], f32, tag="pst")
            for b in range(BL):
                nc.tensor.matmul(cp[:, b:b + 1],
                                 lhsT=enc_sb[b][:, c * 128:(c + 1) * 128],
                                 rhs=wT[:, b:b + 1],
                                 start=True, stop=True)
            nc.vector.tensor_copy(cxT_sb[:, c * BL:(c + 1) * BL], cp[:])
            nc.sync.dma_start(cxT_o[c * 128:(c + 1) * 128, :].bitcast(f32r),
                              cxT_sb[:, c * BL:(c + 1) * BL])

        # ---- transpose emb and h(=q) to [k, b] chunks for GRU lhsT ----
        embT = acts.tile([128, KH * BL], f32r, tag="embT")
        hT = acts.tile([128, KH * BL], f32r, tag="hT")
        for c in range(KH):
            p1 = ps.tile([128, BL], f32, tag="pst")
            nc.tensor.transpose(p1[:], emb_sb[:, c * 128:(c + 1) * 128],
                                ident[:BL, :BL])
            nc.vector.tensor_copy(embT[:, c * BL:(c + 1) * BL], p1[:])
            p2 = ps.tile([128, BL], f32, tag="pst")
            nc.tensor.transpose(p2[:], q_sb[:, c * 128:(c + 1) * 128],
                                ident[:BL, :BL])
            nc.vector.tensor_copy(hT[:, c * BL:(c + 1) * BL], p2[:])

        # ---- GRU gates: gx = x @ W_ih^T + b_ih ; gh = h @ W_hh^T + b_hh ----
        gx = psg.tile([BL, G3], f32, tag="gx")
        gh = psg.tile([BL, G3], f32, tag="gh")
        for c in range(3):
            nc.tensor.matmul(gx[:, c * 512:(c + 1) * 512], lhsT=ones18[:],
                             rhs=bih_sb[:, c * 512:(c + 1) * 512],
                             start=True, stop=False)
            nc.tensor.matmul(gh[:, c * 512:(c + 1) * 512], lhsT=ones18[:],
                             rhs=bhh_sb[:, c * 512:(c + 1) * 512],
                             start=True, stop=False)
        for k in range(KX):
            src = embT if k < KH else cxT_sb
            kk = k if k < KH else k - KH
            lhs = src[:, kk * BL:(kk + 1) * BL]
            for c in range(3):
                nc.tensor.matmul(
                    gx[:, c * 512:(c + 1) * 512],
                    lhsT=lhs,
                    rhs=wih_sb[:, k * G3 + c * 512:k * G3 + (c + 1) * 512],
                    start=False, stop=(k == KX - 1))
        for k in range(KH):
            lhs = hT[:, k * BL:(k + 1) * BL]
            for c in range(3):
                nc.tensor.matmul(
                    gh[:, c * 512:(c + 1) * 512],
                    lhsT=lhs,
                    rhs=whh_sb[:, k * G3 + c * 512:k * G3 + (c + 1) * 512],
                    start=False, stop=(k == KH - 1))

        # ---- gate nonlinearities (PyTorch order r, z, n) ----
        gx_sb = acts.tile([BL, G3], f32, tag="gxsb")
        nc.vector.tensor_copy(gx_sb[:], gx[:])
        r_in = acts.tile([BL, H], f32, tag="rin")
        nc.vector.tensor_add(r_in[:], gx_sb[:, 0:H], gh[:, 0:H])
        r_sb = acts.tile([BL, H], f32, tag="rsb")
        nc.scalar.activation(r_sb[:], r_in[:], AT.Sigmoid)
        z_in = acts.tile([BL, H], f32, tag="zin")
        nc.vector.tensor_add(z_in[:], gx_sb[:, H:2 * H], gh[:, H:2 * H])
        z_sb = acts.tile([BL, H], f32, tag="zsb")
        nc.scalar.activation(z_sb[:], z_in[:], AT.Sigmoid)
        nmul = acts.tile([BL, H], f32, tag="nmul")
        nc.vector.tensor_mul(nmul[:], r_sb[:], gh[:, 2 * H:3 * H])
        n_in = acts.tile([BL, H], f32, tag="nin")
        nc.vector.tensor_add(n_in[:], nmul[:], gx_sb[:, 2 * H:3 * H])
        n_sb = acts.tile([BL, H], f32, tag="nsb")
        nc.scalar.activation(n_sb[:], n_in[:], AT.Tanh)
        # h_new = (1-z)*n + z*h = n + z*(h-n)
        d_sb = acts.tile([BL, H], f32, tag="dsb")
        nc.vector.tensor_sub(d_sb[:], q_sb[:], n_sb[:])
        zd = acts.tile([BL, H], f32, tag="zd")
        nc.vector.tensor_mul(zd[:], z_sb[:], d_sb[:])
        hnew = acts.tile([BL, H], f32, tag="hnew")
        nc.vector.tensor_add(hnew[:], n_sb[:], zd[:])
        nc.sync.dma_start(hn_o[:], hnew[:])

    nc.compile()
    return nc


def _build_l2():
    from contextlib import ExitStack
    from concourse import bacc, tile, mybir

    f32 = mybir.dt.float32
    f32r = mybir.dt.float32r

    nc = bacc.Bacc("TRN2", target_bir_lowering=False, debug=False,
                   num_devices=NC)
    x2 = nc.dram_tensor("x2", [EMB + H, B], f32r, kind="ExternalInput").ap()
    wt = nc.dram_tensor("wt", [EMB + H, VL], f32r, kind="ExternalInput").ap()
    bo = nc.dram_tensor("bo", [1, VL], f32r, kind="ExternalInput").ap()
    lg = nc.dram_tensor("lg", [B, VL], f32, kind="ExternalOutput").ap()

    NNB = VL // NB
    with tile.TileContext(nc) as tc, ExitStack() as ctx:
        const = ctx.enter_context(tc.tile_pool(name="const", bufs=1))
        xp = ctx.enter_context(tc.tile_pool(name="xp", bufs=1))
        wp = ctx.enter_context(tc.tile_pool(name="wp", bufs=1))
        op = ctx.enter_context(tc.tile_pool(name="op", bufs=4))
        psp = ctx.enter_context(tc.tile_pool(name="psp", bufs=1, space="PSUM"))

        onesf = const.tile([1, B], f32, tag="onesf")
        nc.vector.memset(onesf[:], 1.0)
        ones = const.tile([1, B], f32r, tag="ones")
        nc.vector.tensor_copy(ones[:], onesf[:])
        bo_sb = const.tile([1, VL], f32r, tag="bo")
        nc.sync.dma_start(bo_sb[:], bo[:])
        x2_sb = xp.tile([128, KX * B], f32r, tag="x2")
        for k in range(KX):
            nc.sync.dma_start(x2_sb[:, k * B:(k + 1) * B],
                              x2[k * 128:(k + 1) * 128, :])
        wt_sb = []
        for k in range(KX):
            t = wp.tile([128, VL], f32r, tag=f"wt{k}", name=f"wt{k}")
            nc.sync.dma_start(t[:], wt[k * 128:(k + 1) * 128, :])
            wt_sb.append(t)

        lgp = [psp.tile([B, NB], f32, tag=f"lg{nb}", name=f"lgp{nb}")
               for nb in range(NNB)]
        for nb in range(NNB):
            nc.tensor.matmul(lgp[nb][:], lhsT=ones[:],
                             rhs=bo_sb[:, nb * NB:(nb + 1) * NB],
                             start=True, stop=False)
        for k in range(KX):
            for nb in range(NNB):
                nc.tensor.matmul(lgp[nb][:],
                                 lhsT=x2_sb[:, k * B:(k + 1) * B],
                                 rhs=wt_sb[k][:, nb * NB:(nb + 1) * NB],
                                 start=False, stop=(k == KX - 1))
                if k == KX - 1:
                    o = op.tile([B, NB], f32, tag="o", name=f"o{nb}")
                    nc.vector.tensor_copy(o[:], lgp[nb][:])
                    nc.sync.dma_start(lg[:, nb * NB:(nb + 1) * NB], o[:])

    nc.compile()
    return nc


last_exec_times = []


def _install_ntff_hook():
    """Shim antenv.axon_hooks so trace=True captures NTFF under axon."""
    import sys, types
    try:
        import antenv.axon_hooks  # noqa: F401
        return
    except ImportError:
        pass
    try:
        import antenv
        from trn_agent_boot.trn_boot import _ntff_profile_via_ctypes
        mod = types.ModuleType("antenv.axon_hooks")
        _store = {}
        mod.set_axon_ntff_profile_hook = lambda h: _store.update(h=h)
        mod.get_axon_ntff_profile_hook = lambda: _store.get("h")
        sys.modules["antenv.axon_hooks"] = mod
        antenv.axon_hooks = mod
        mod.set_axon_ntff_profile_hook(
            _ntff_profile_via_ctypes("/opt/axon/libaxon_pjrt.so"))
    except Exception:
        pass


def _run(nc, in_maps):
    from concourse import bass_utils
    trace = bool(int(os.environ.get("BASSDEC_TRACE", "0")))
    if trace:
        _install_ntff_hook()
    res = bass_utils.run_bass_kernel_spmd(
        nc, in_maps, core_ids=list(range(NC)), trace=trace)
    if trace:
        last_exec_times.append(res.exec_time_ns)
    return res.results


def kernel(input_token, hidden, enc_out, src_mask, emb_table,
           W_ih, W_hh, b_ih, b_hh, W_out, b_out):
    f = np.float32
    input_token = np.asarray(input_token)
    hidden = np.asarray(hidden, f)
    enc_out = np.ascontiguousarray(np.asarray(enc_out, f))
    src_mask_f = np.asarray(src_mask).astype(f)
    emb_table = np.asarray(emb_table, f)
    W_ih = np.asarray(W_ih, f)
    W_hh = np.asarray(W_hh, f)
    b_ih = np.asarray(b_ih, f)
    b_hh = np.asarray(b_hh, f)
    W_out = np.asarray(W_out, f)
    b_out = np.asarray(b_out, f)

    l1 = _get("l1", _build_l1)
    l2 = _get("l2", _build_l2)

    tokens = input_token.astype(np.int64)
    emb_rows = np.ascontiguousarray(emb_table[tokens])          # [64, 512]
    wihT = np.ascontiguousarray(W_ih.T)                          # [1024, 1536]
    whhT = np.ascontiguousarray(W_hh.T)                          # [512, 1536]
    bih_r = np.ascontiguousarray(b_ih.reshape(1, G3))
    bhh_r = np.ascontiguousarray(b_hh.reshape(1, G3))
    q = hidden[0]                                                # [64, 512]

    in_maps1 = []
    for c in range(NC):
        sl = slice(c * BL, (c + 1) * BL)
        in_maps1.append({
            "enc": np.ascontiguousarray(enc_out[sl]),
            "q": np.ascontiguousarray(q[sl]),
            "msk": np.ascontiguousarray(src_mask_f[sl]),
            "emb": np.ascontiguousarray(emb_rows[sl]),
            "wih": wihT, "whh": whhT, "bih": bih_r, "bhh": bhh_r,
        })
    res1 = _run(l1, in_maps1)

    h_new = np.concatenate([res1[c]["hn"] for c in range(NC)], axis=0)
    ctx = np.concatenate([res1[c]["cxT"].T for c in range(NC)], axis=0)
    x2 = np.concatenate([h_new, ctx], axis=1)                    # [64, 1024]
    x2T = np.ascontiguousarray(x2.T)                             # [1024, 64]
    W_outT = np.ascontiguousarray(W_out.T)                       # [1024, 32000]

    in_maps2 = []
    for c in range(NC):
        vs = slice(c * VL, (c + 1) * VL)
        in_maps2.append({
            "x2": x2T,
            "wt": np.ascontiguousarray(W_outT[:, vs]),
            "bo": np.ascontiguousarray(b_out[vs].reshape(1, VL)),
        })
    res2 = _run(l2, in_maps2)

    logits = np.concatenate([res2[c]["lg"] for c in range(NC)], axis=1)
    return logits.astype(f), h_new[None].astype(f)


# revision 14
# speedup vs baseline: 1.4633x; 1.0316x over previous
"""Trainium2 Bass kernel for nn_Decoder (attention GRU decoder step).

Strategy (8 NeuronCores, zero collectives — collectives cost ~75us+ in this
environment, far above their data cost):

  Launch 1 (batch-parallel, 8 batches/core):
      dot-attention (scores via DVE mul+reduce, softmax, context via PE)
      + GRU cell (PE matmuls, gates on DVE/ACT).
      Outputs per core: h_new rows [8, 512], context^T [512, 8].
  Host: assembles x2h^T = [h_new; context]^T  (tiny: 256 KB).
  Launch 2 (vocab-parallel, 4000 vocab rows/core):
      logits slice [64, 4000] = x2h @ W_out_slice^T + b_out_slice,
      streamed fp32r matmuls against the 16.4 MB/core weight slice.

All heavy compute runs on device; the host only shards/reassembles and
pre-transposes weights (layout prep, done once per call).
"""

import os
import numpy as np

B, S, H, EMB, V = 64, 128, 512, 512, 32000
NC = 8
BL = B // NC          # 8 local batches per core
VL = V // NC          # 4000 vocab rows per core
NB = 500              # psum bank chunk of the vocab slice (8 * 500 = 4000)
KX = (EMB + H) // 128  # 8 k-chunks of x
KH = H // 128          # 4 k-chunks of h
G3 = 3 * H             # 1536 gate units

_cache = {}


def _get(key, builder):
    if key not in _cache:
        _cache[key] = builder()
    return _cache[key]


def _build_l1():
    from contextlib import ExitStack
    from concourse import bacc, tile, mybir
    from concourse.masks import make_identity

    f32 = mybir.dt.float32
    f32r = mybir.dt.float32r
    AT = mybir.ActivationFunctionType
    OP = mybir.AluOpType
    AX = mybir.AxisListType

    nc = bacc.Bacc("TRN2", target_bir_lowering=False, debug=False,
                   num_devices=NC)
    enc = nc.dram_tensor("enc", [BL, S, H], f32r, kind="ExternalInput").ap()
    q_in = nc.dram_tensor("q", [BL, H], f32, kind="ExternalInput").ap()
    msk = nc.dram_tensor("msk", [BL, S], f32, kind="ExternalInput").ap()
    emb = nc.dram_tensor("emb", [BL, EMB], f32, kind="ExternalInput").ap()
    wih = nc.dram_tensor("wih", [EMB + H, G3], f32r, kind="ExternalInput").ap()
    whh = nc.dram_tensor("whh", [H, G3], f32r, kind="ExternalInput").ap()
    bih = nc.dram_tensor("bih", [1, G3], f32r, kind="ExternalInput").ap()
    bhh = nc.dram_tensor("bhh", [1, G3], f32r, kind="ExternalInput").ap()
    hn_o = nc.dram_tensor("hn", [BL, H], f32, kind="ExternalOutput").ap()
    cxT_o = nc.dram_tensor("cxT", [H, BL], f32, kind="ExternalOutput").ap()

    with tile.TileContext(nc) as tc, ExitStack() as ctx:
        const = ctx.enter_context(tc.tile_pool(name="const", bufs=1))
        acts = ctx.enter_context(tc.tile_pool(name="acts", bufs=1))
        wp = ctx.enter_context(tc.tile_pool(name="wp", bufs=1))
        bc = ctx.enter_context(tc.tile_pool(name="bc", bufs=2))
        ps = ctx.enter_context(tc.tile_pool(name="ps", bufs=2, space="PSUM"))
        psg = ctx.enter_context(tc.tile_pool(name="psg", bufs=1, space="PSUM"))

        ident = const.tile([128, 128], f32, tag="ident")
        make_identity(nc, ident[:])
        ones18f = const.tile([1, BL], f32, tag="ones18f")
        nc.vector.memset(ones18f[:], 1.0)
        ones18 = const.tile([1, BL], f32r, tag="ones18")
        nc.vector.tensor_copy(ones18[:], ones18f[:])

        # -- DMA order: tiny activation loads first, GRU weights last --
        qcat = acts.tile([1, BL * H], f32, tag="qcat")
        nc.sync.dma_start(qcat[:], q_in.rearrange("a b -> (a b)")[None, :])
        q_sb = acts.tile([BL, H], f32, tag="q")
        nc.sync.dma_start(q_sb[:], q_in[:])
        msk_sb = acts.tile([BL, S], f32, tag="msk")
        nc.sync.dma_start(msk_sb[:], msk[:])
        emb_sb = acts.tile([BL, EMB], f32, tag="emb")
        nc.sync.dma_start(emb_sb[:], emb[:])
        bih_sb = const.tile([1, G3], f32r, tag="bih")
        nc.sync.dma_start(bih_sb[:], bih[:])
        bhh_sb = const.tile([1, G3], f32r, tag="bhh")
        nc.sync.dma_start(bhh_sb[:], bhh[:])
        enc_sb = []
        for b in range(BL):
            t = acts.tile([S, H], f32r, tag=f"enc{b}", name=f"enc{b}")
            nc.sync.dma_start(t[:], enc[b, :, :])
            enc_sb.append(t)
        wih_sb = wp.tile([128, KX * G3], f32r, tag="wih")
        for k in range(KX):
            nc.sync.dma_start(wih_sb[:, k * G3:(k + 1) * G3],
                              wih[k * 128:(k + 1) * 128, :])
        whh_sb = wp.tile([128, KH * G3], f32r, tag="whh")
        for k in range(KH):
            nc.sync.dma_start(whh_sb[:, k * G3:(k + 1) * G3],
                              whh[k * 128:(k + 1) * 128, :])

        # -- transposes of emb and h(=q) -> [k, b] chunks (PE, early) --
        embT = acts.tile([128, KH * BL], f32r, tag="embT")
        hT = acts.tile([128, KH * BL], f32r, tag="hT")
        for c in range(KH):
            p1 = ps.tile([128, BL], f32, tag="pst", name=f"tpe{c}")
            nc.tensor.transpose(p1[:], emb_sb[:, c * 128:(c + 1) * 128],
                                ident[:BL, :BL])
            nc.vector.tensor_copy(embT[:, c * BL:(c + 1) * BL], p1[:])
            p2 = ps.tile([128, BL], f32, tag="pst", name=f"tph{c}")
            nc.tensor.transpose(p2[:], q_sb[:, c * 128:(c + 1) * 128],
                                ident[:BL, :BL])
            nc.vector.tensor_copy(hT[:, c * BL:(c + 1) * BL], p2[:])

        # -- GRU gate psums; gh and the emb half of gx can run early --
        gx = psg.tile([BL, G3], f32, tag="gx")
        gh = psg.tile([BL, G3], f32, tag="gh")
        for c in range(3):
            nc.tensor.matmul(gx[:, c * 512:(c + 1) * 512], lhsT=ones18[:],
                             rhs=bih_sb[:, c * 512:(c + 1) * 512],
                             start=True, stop=False)
            nc.tensor.matmul(gh[:, c * 512:(c + 1) * 512], lhsT=ones18[:],
                             rhs=bhh_sb[:, c * 512:(c + 1) * 512],
                             start=True, stop=False)
        for k in range(KH):
            lhs = hT[:, k * BL:(k + 1) * BL]
            for c in range(3):
                nc.tensor.matmul(
                    gh[:, c * 512:(c + 1) * 512], lhsT=lhs,
                    rhs=whh_sb[:, k * G3 + c * 512:k * G3 + (c + 1) * 512],
                    start=False, stop=(k == KH - 1))
        for k in range(KH):  # emb half of x
            lhs = embT[:, k * BL:(k + 1) * BL]
            for c in range(3):
                nc.tensor.matmul(
                    gx[:, c * 512:(c + 1) * 512], lhsT=lhs,
                    rhs=wih_sb[:, k * G3 + c * 512:k * G3 + (c + 1) * 512],
                    start=False, stop=False)

        # -- scores^T [128s, 8b]: q broadcast + fused mul-reduce on DVE --
        scT = acts.tile([S, BL], f32, tag="scT")
        junk = bc.tile([S, H], f32, tag="junk")
        for b in range(BL):
            qb = bc.tile([S, H], f32, tag="qb")
            nc.gpsimd.partition_broadcast(qb[:], qcat[0:1, b * H:(b + 1) * H])
            nc.vector.scalar_tensor_tensor(
                out=junk[:], in0=enc_sb[b][:].bitcast(f32), scalar=1.0,
                in1=qb[:], op0=OP.mult, op1=OP.mult,
                accum_out=scT[:, b:b + 1])

        # -- transpose scores -> [8b, 128s], mask, softmax --
        sc_ps = ps.tile([BL, S], f32, tag="pst")
        nc.tensor.transpose(sc_ps[:], scT[:], ident[:])
        sc = acts.tile([BL, S], f32, tag="sc")
        nc.vector.tensor_copy(sc[:], sc_ps[:])
        t1 = acts.tile([BL, S], f32, tag="t1")
        nc.vector.tensor_mul(t1[:], sc[:], msk_sb[:])
        t2 = acts.tile([BL, S], f32, tag="t2")
        nc.vector.tensor_scalar(out=t2[:], in0=msk_sb[:], scalar1=1.0,
                                scalar2=100000.0, op0=OP.subtract,
                                op1=OP.mult)
        tm = acts.tile([BL, S], f32, tag="tm")
        nc.vector.tensor_add(tm[:], t1[:], t2[:])
        rmax = acts.tile([BL, 1], f32, tag="rmax")
        nc.vector.reduce_max(rmax[:], tm[:], axis=AX.X)
        nmax = acts.tile([BL, 1], f32, tag="nmax")
        nc.vector.tensor_scalar_mul(nmax[:], rmax[:], -1.0)
        esb = acts.tile([BL, S], f32, tag="esb")
        rsum = acts.tile([BL, 1], f32, tag="rsum")
        nc.scalar.activation(esb[:], tm[:], AT.Exp, bias=nmax[:, 0:1],
                             scale=1.0, accum_out=rsum[:])
        rinv = acts.tile([BL, 1], f32, tag="rinv")
        nc.vector.reciprocal(rinv[:], rsum[:])
        wgt = acts.tile([BL, S], f32, tag="wgt")
        nc.vector.tensor_scalar_mul(wgt[:], esb[:], rinv[:, 0:1])
        wT_ps = ps.tile([S, BL], f32, tag="pst")
        nc.tensor.transpose(wT_ps[:], wgt[:], ident[:BL, :BL])
        wT = acts.tile([S, BL], f32r, tag="wT")
        nc.vector.tensor_copy(wT[:], wT_ps[:])

        # -- context rows [1, 512] per batch (f32r, N=512), then gather
        #    rows via SBUF->SBUF DMA and transpose to [k, b] chunks --
        ctx_rows = acts.tile([BL, H], f32, tag="ctxrows")
        for b in range(BL):
            cr_ps = ps.tile([1, H], f32, tag="pst", name=f"crps{b}")
            nc.tensor.matmul(cr_ps[:], lhsT=wT[:, b:b + 1],
                             rhs=enc_sb[b][:], start=True, stop=True)
            cr_sb = bc.tile([1, H], f32, tag="crsb", name=f"crsb{b}")
            nc.vector.tensor_copy(cr_sb[:], cr_ps[:])
            nc.sync.dma_start(ctx_rows[b:b + 1, :], cr_sb[:])
        cxT_sb = acts.tile([128, KH * BL], f32r, tag="cxT")
        for c in range(KH):
            cp = ps.tile([128, BL], f32, tag="pst", name=f"cps{c}")
            nc.tensor.transpose(cp[:], ctx_rows[:, c * 128:(c + 1) * 128],
                                ident[:BL, :BL])
            nc.vector.tensor_copy(cxT_sb[:, c * BL:(c + 1) * BL], cp[:])
            nc.sync.dma_start(cxT_o[c * 128:(c + 1) * 128, :].bitcast(f32r),
                              cxT_sb[:, c * BL:(c + 1) * BL])

        # -- ctx half of gx --
        for k in range(KH):
            lhs = cxT_sb[:, k * BL:(k + 1) * BL]
            kk = k + KH
            for c in range(3):
                nc.tensor.matmul(
                    gx[:, c * 512:(c + 1) * 512], lhsT=lhs,
                    rhs=wih_sb[:, kk * G3 + c * 512:kk * G3 + (c + 1) * 512],
                    start=False, stop=(k == KH - 1))

        # -- gate nonlinearities (PyTorch order r, z, n) --
        gx_sb = acts.tile([BL, G3], f32, tag="gxsb")
        nc.vector.tensor_copy(gx_sb[:], gx[:])
        r_in = acts.tile([BL, H], f32, tag="rin")
        nc.vector.tensor_add(r_in[:], gx_sb[:, 0:H], gh[:, 0:H])
        r_sb = acts.tile([BL, H], f32, tag="rsb")
        nc.scalar.activation(r_sb[:], r_in[:], AT.Sigmoid)
        z_in = acts.tile([BL, H], f32, tag="zin")
        nc.vector.tensor_add(z_in[:], gx_sb[:, H:2 * H], gh[:, H:2 * H])
        z_sb = acts.tile([BL, H], f32, tag="zsb")
        nc.scalar.activation(z_sb[:], z_in[:], AT.Sigmoid)
        nmul = acts.tile([BL, H], f32, tag="nmul")
        nc.vector.tensor_mul(nmul[:], r_sb[:], gh[:, 2 * H:3 * H])
        n_in = acts.tile([BL, H], f32, tag="nin")
        nc.vector.tensor_add(n_in[:], nmul[:], gx_sb[:, 2 * H:3 * H])
        n_sb = acts.tile([BL, H], f32, tag="nsb")
        nc.scalar.activation(n_sb[:], n_in[:], AT.Tanh)
        d_sb = acts.tile([BL, H], f32, tag="dsb")
        nc.vector.tensor_sub(d_sb[:], q_sb[:], n_sb[:])
        zd = acts.tile([BL, H], f32, tag="zd")
        nc.vector.tensor_mul(zd[:], z_sb[:], d_sb[:])
        hnew = acts.tile([BL, H], f32, tag="hnew")
        nc.vector.tensor_add(hnew[:], n_sb[:], zd[:])
        nc.sync.dma_start(hn_o[:], hnew[:])

    nc.compile()
    return nc


def _build_l2():
    from contextlib import ExitStack
    from concourse import bacc, tile, mybir

    f32 = mybir.dt.float32
    f32r = mybir.dt.float32r

    nc = bacc.Bacc("TRN2", target_bir_lowering=False, debug=False,
                   num_devices=NC)
    x2 = nc.dram_tensor("x2", [EMB + H, B], f32r, kind="ExternalInput").ap()
    wt = nc.dram_tensor("wt", [EMB + H, VL], f32r, kind="ExternalInput").ap()
    bo = nc.dram_tensor("bo", [1, VL], f32r, kind="ExternalInput").ap()
    lg = nc.dram_tensor("lg", [B, VL], f32, kind="ExternalOutput").ap()

    NNB = VL // NB
    with tile.TileContext(nc) as tc, ExitStack() as ctx:
        const = ctx.enter_context(tc.tile_pool(name="const", bufs=1))
        xp = ctx.enter_context(tc.tile_pool(name="xp", bufs=1))
        wp = ctx.enter_context(tc.tile_pool(name="wp", bufs=1))
        op = ctx.enter_context(tc.tile_pool(name="op", bufs=4))
        psp = ctx.enter_context(tc.tile_pool(name="psp", bufs=1, space="PSUM"))

        onesf = const.tile([1, B], f32, tag="onesf")
        nc.vector.memset(onesf[:], 1.0)
        ones = const.tile([1, B], f32r, tag="ones")
        nc.vector.tensor_copy(ones[:], onesf[:])
        bo_sb = const.tile([1, VL], f32r, tag="bo")
        nc.sync.dma_start(bo_sb[:], bo[:])
        x2_sb = xp.tile([128, KX * B], f32r, tag="x2")
        for k in range(KX):
            nc.sync.dma_start(x2_sb[:, k * B:(k + 1) * B],
                              x2[k * 128:(k + 1) * 128, :])
        wt_sb = []
        for k in range(KX):
            t = wp.tile([128, VL], f32r, tag=f"wt{k}", name=f"wt{k}")
            nc.sync.dma_start(t[:], wt[k * 128:(k + 1) * 128, :])
            wt_sb.append(t)

        lgp = [psp.tile([B, NB], f32, tag=f"lg{nb}", name=f"lgp{nb}")
               for nb in range(NNB)]
        for nb in range(NNB):
            nc.tensor.matmul(lgp[nb][:], lhsT=ones[:],
                             rhs=bo_sb[:, nb * NB:(nb + 1) * NB],
                             start=True, stop=False)
        for k in range(KX):
            for nb in range(NNB):
                nc.tensor.matmul(lgp[nb][:],
                                 lhsT=x2_sb[:, k * B:(k + 1) * B],
                                 rhs=wt_sb[k][:, nb * NB:(nb + 1) * NB],
                                 start=False, stop=(k == KX - 1))
                if k == KX - 1:
                    o = op.tile([B, NB], f32, tag="o", name=f"o{nb}")
                    nc.vector.tensor_copy(o[:], lgp[nb][:])
                    nc.sync.dma_start(lg[:, nb * NB:(nb + 1) * NB], o[:])

    nc.compile()
    return nc


last_exec_times = []


def _install_ntff_hook():
    """Shim antenv.axon_hooks so trace=True captures NTFF under axon."""
    import sys, types
    try:
        import antenv.axon_hooks  # noqa: F401
        return
    except ImportError:
        pass
    try:
        import antenv
        from trn_agent_boot.trn_boot import _ntff_profile_via_ctypes
        mod = types.ModuleType("antenv.axon_hooks")
        _store = {}
        mod.set_axon_ntff_profile_hook = lambda h: _store.update(h=h)
        mod.get_axon_ntff_profile_hook = lambda: _store.get("h")
        sys.modules["antenv.axon_hooks"] = mod
        antenv.axon_hooks = mod
        mod.set_axon_ntff_profile_hook(
            _ntff_profile_via_ctypes("/opt/axon/libaxon_pjrt.so"))
    except Exception:
        pass


def _run(nc, in_maps):
    from concourse import bass_utils
    trace = bool(int(os.environ.get("BASSDEC_TRACE", "0")))
    if trace:
        _install_ntff_hook()
    res = bass_utils.run_bass_kernel_spmd(
        nc, in_maps, core_ids=list(range(NC)), trace=trace)
    if trace:
        last_exec_times.append(res.exec_time_ns)
    return res.results


def kernel(input_token, hidden, enc_out, src_mask, emb_table,
           W_ih, W_hh, b_ih, b_hh, W_out, b_out):
    f = np.float32
    input_token = np.asarray(input_token)
    hidden = np.asarray(hidden, f)
    enc_out = np.ascontiguousarray(np.asarray(enc_out, f))
    src_mask_f = np.asarray(src_mask).astype(f)
    emb_table = np.asarray(emb_table, f)
    W_ih = np.asarray(W_ih, f)
    W_hh = np.asarray(W_hh, f)
    b_ih = np.asarray(b_ih, f)
    b_hh = np.asarray(b_hh, f)
    W_out = np.asarray(W_out, f)
    b_out = np.asarray(b_out, f)

    l1 = _get("l1", _build_l1)
    l2 = _get("l2", _build_l2)

    tokens = input_token.astype(np.int64)
    emb_rows = np.ascontiguousarray(emb_table[tokens])          # [64, 512]
    wihT = np.ascontiguousarray(W_ih.T)                          # [1024, 1536]
    whhT = np.ascontiguousarray(W_hh.T)                          # [512, 1536]
    bih_r = np.ascontiguousarray(b_ih.reshape(1, G3))
    bhh_r = np.ascontiguousarray(b_hh.reshape(1, G3))
    q = hidden[0]                                                # [64, 512]

    in_maps1 = []
    for c in range(NC):
        sl = slice(c * BL, (c + 1) * BL)
        in_maps1.append({
            "enc": np.ascontiguousarray(enc_out[sl]),
            "q": np.ascontiguousarray(q[sl]),
            "msk": np.ascontiguousarray(src_mask_f[sl]),
            "emb": np.ascontiguousarray(emb_rows[sl]),
            "wih": wihT, "whh": whhT, "bih": bih_r, "bhh": bhh_r,
        })
    res1 = _run(l1, in_maps1)

    h_new = np.concatenate([res1[c]["hn"] for c in range(NC)], axis=0)
    ctx = np.concatenate([res1[c]["cxT"].T for c in range(NC)], axis=0)
    x2 = np.concatenate([h_new, ctx], axis=1)                    # [64, 1024]
    x2T = np.ascontiguousarray(x2.T)                             # [1024, 64]
    W_outT = np.ascontiguousarray(W_out.T)                       # [1024, 32000]

    in_maps2 = []
    for c in range(NC):
        vs = slice(c * VL, (c + 1) * VL)
        in_maps2.append({
            "x2": x2T,
            "wt": np.ascontiguousarray(W_outT[:, vs]),
            "bo": np.ascontiguousarray(b_out[vs].reshape(1, VL)),
        })
    res2 = _run(l2, in_maps2)

    logits = np.concatenate([res2[c]["lg"] for c in range(NC)], axis=1)
    return logits.astype(f), h_new[None].astype(f)
